# revision 1
# baseline (speedup 1.0000x reference)
"""ChannelAttention (B,D,H,W,C = 4,8,32,32,512; 8 heads, head_dim 64) on 8
Trainium2 NeuronCores, Bass/Tile SPMD. Fully data-parallel: zero cross-core
communication.

Sharding: the 32768 tokens (B * D*H*W) are split 8 ways -> 4096 output tokens
per core; cores (2j, 2j+1) handle the two halves of batch j. Channel
attention needs the per-head 64x64 k^T v Gram matrix over ALL of a batch's
tokens, so each core redundantly computes k|v for its whole batch (8192
tokens; its own half ordered first in its xT input). This duplicated k|v work
(~55us of PE) is cheaper and far more robust than any cross-core reduction
(a ncfw AllReduce costs ~70us fixed + a 67us start barrier).

Schedule per core:
  pass A   : stream xT chunks (16 = both halves), k|v = x @ Wkv^T (k scale
             folded in on host), accumulate per-head-pair k^T v into PSUM
             (head-pair x head-quad blocking so fp32r matmuls hit the N>=256
             full-rate mode). HAM warm-up keeper matmuls run during the
             initial DMA fill.
  softmax  : pack 8 64x64 blocks, rowwise softmax over e on [128, 4, 64]
             (DVE reduce/reciprocal, ACT exp) -- overlapped with
  pass B   : qT = Wq @ x^T for the core's own 4096 tokens (re-streams x).
  phase 2  : PE-transpose probs into block-diagonal pair lhsT, out = attnT @
             qT, proj y = out @ Wproj^T (+bias via DVE broadcast add),
             software-pipelined one chunk ahead.

Numerics: all matmuls in float32r (fp32 storage, reduced-precision PE
multiply, ~13-bit effective mantissa) with fp32 PSUM accumulation; softmax in
fp32. End-to-end L2 relative error vs the fp32 reference: ~1.0e-3.
"""

import os
import numpy as np
from contextlib import ExitStack

import concourse.bass as bass
import concourse.mybir as mybir
import concourse.tile as tile
from concourse import bacc
from concourse.bass_utils import run_bass_kernel_spmd
from concourse.masks import make_identity

B, D, H, W, C = 4, 8, 32, 32, 512
NUM_HEADS = 8
HEAD_DIM = C // NUM_HEADS
SCALE = HEAD_DIM ** -0.5
N_TOK = B * D * H * W
N_CORES = 8
N_LOC = N_TOK // N_CORES
CHUNK = 512
N_CHUNKS = N_LOC // CHUNK
TT = 128
T_PER_CHUNK = CHUNK // TT
N_CI = C // 128
N_PAIRS = NUM_HEADS // 2

f32 = mybir.dt.float32
f32r = mybir.dt.float32r

N_KEEP_START = 24
N_KEEP_MID = 16

_NC_CACHE = None


def build_nc():
    nc = bacc.Bacc(num_devices=N_CORES)

    xT = nc.declare_dram_parameter("xT", [C, 2 * N_LOC], f32r, isOutput=False)
    wq = nc.declare_dram_parameter("wq", [C, C], f32r, isOutput=False)
    wkv = nc.declare_dram_parameter("wkv", [C, 2 * C], f32r, isOutput=False)
    wp = nc.declare_dram_parameter("wp", [C, C], f32r, isOutput=False)
    bp = nc.declare_dram_parameter("bp", [1, C], f32r, isOutput=False)
    y = nc.declare_dram_parameter("y", [N_LOC, C], f32, isOutput=True)

    xT_v = xT.rearrange("(a p) n -> p a n", p=128)
    wq_v = wq.rearrange("(a p) f -> p a f", p=128)
    wkv_v = wkv.rearrange("(a p) f -> p a f", p=128)
    wp_v = wp.rearrange("(a p) f -> p a f", p=128)

    with tile.TileContext(nc) as tc, ExitStack() as ctx:
        const = ctx.enter_context(tc.tile_pool(name="const", bufs=1))
        persist = ctx.enter_context(tc.tile_pool(name="persist", bufs=1))
        sb = ctx.enter_context(tc.tile_pool(name="sb", bufs=2))
        kvp = ctx.enter_context(tc.tile_pool(name="kvp", bufs=4))

        wkv_sb = const.tile([128, N_CI, 2 * C], f32r)
        nc.sync.dma_start(wkv_sb[:], wkv_v[:])
        ones_f32 = const.tile([1, 128], f32)
        nc.vector.memset(ones_f32[:], 1.0)
        ones_sb = const.tile([1, 128], f32r)
        nc.vector.tensor_copy(ones_sb[:], ones_f32[:])
        zrow_f32 = const.tile([1, 512], f32)
        nc.vector.memset(zrow_f32[:], 0.0)
        zrow_sb = const.tile([1, 512], f32r)
        nc.vector.tensor_copy(zrow_sb[:], zrow_f32[:])
        ident = const.tile([128, 128], f32)
        make_identity(nc, ident[:])

        qT_all = persist.tile([128, N_PAIRS, N_CHUNKS, CHUNK], f32r)
        red_sb = persist.tile([128, N_PAIRS, 64], f32)

        # ---------------- pass A: k|v + attn partial accumulation ----------
        with (
            tc.tile_pool(name="ps_kv", bufs=2, space="PSUM") as ps_kv,
            tc.tile_pool(name="ps_at", bufs=1, space="PSUM") as ps_at,
            tc.tile_pool(name="ps_q", bufs=2, space="PSUM") as ps_q,
        ):
            attn_ps = ps_at.tile([128, N_PAIRS, 256], f32)
            # bank-wide has_written seed + HAM warm-up
            for i in range(max(2, N_KEEP_START)):
                bank = i % 2
                nc.tensor.matmul(
                    attn_ps[:, 2 * bank:2 * bank + 2, :].rearrange("p a e -> p (a e)"),
                    ones_sb[:], zrow_sb[:],
                    start=(i < 2), stop=False, skip_group_check=True,
                )

            for c in range(2 * N_CHUNKS):
                xt = sb.tile([128, N_CI, CHUNK], f32r, tag="xt")
                nc.sync.dma_start(xt[:], xT_v[:, :, c * CHUNK:(c + 1) * CHUNK])

                kv_tiles = []
                for s in range(T_PER_CHUNK):
                    kv_ps = ps_kv.tile([128, 2 * C], f32, tag="kv")
                    for h in range(2):
                        for k in range(N_CI):
                            nc.tensor.matmul(
                                kv_ps[:, h * C:(h + 1) * C],
                                xt[:, k, s * TT:(s + 1) * TT],
                                wkv_sb[:, k, h * C:(h + 1) * C],
                                start=(k == 0), stop=(k == N_CI - 1),
                            )
                    kv_sb = kvp.tile([128, 2 * C], f32r, tag="kvsb")
                    nc.vector.tensor_copy(kv_sb[:], kv_ps[:])
                    kv_tiles.append(kv_sb)

                for s in range(T_PER_CHUNK):
                    kv_sb = kv_tiles[s]
                    for p in range(N_PAIRS):
                        q4 = p // 2
                        nc.tensor.matmul(
                            attn_ps[:, p, :],
                            kv_sb[:, p * 128:(p + 1) * 128],
                            kv_sb[:, C + q4 * 256:C + (q4 + 1) * 256],
                            start=False,
                            stop=(c == 2 * N_CHUNKS - 1 and s == T_PER_CHUNK - 1),
                            skip_group_check=True,
                        )

            # pack 8 useful 64x64 blocks -> red_sb[d + 64*(h%2), h//2, :]
            for h in range(NUM_HEADS):
                p = h // 2
                row0 = (h % 2) * 64
                col0 = (p % 2) * 128 + row0
                nc.vector.tensor_copy(
                    red_sb[row0:row0 + 64, h // 2, :],
                    attn_ps[row0:row0 + 64, p, col0:col0 + 64],
                )

            # weights for pass B / phase 2 (loaded once pass A's DMAs drain)
            wq_sb = const.tile([128, N_CI, C], f32r)
            nc.sync.dma_start(wq_sb[:], wq_v[:])
            wp_sb = const.tile([128, N_CI, C], f32r)
            nc.sync.dma_start(wp_sb[:], wp_v[:])
            bp_f32 = const.tile([128, C], f32)
            bp_bcast = bass.AP(
                tensor=bp[:].bitcast(f32).tensor,
                offset=0,
                ap=[[0, 128], [1, C]],
            )
            nc.sync.dma_start(bp_f32[:], bp_bcast)

            # ---- softmax over e on [128, pair, 64] (overlaps pass B) ----
            nmax = sb.tile([128, N_PAIRS, 1], f32, tag="nmax")
            nc.vector.reduce_max(nmax[:], red_sb[:], axis=mybir.AxisListType.X, negate=True)
            shifted = sb.tile([128, N_PAIRS, 64], f32, tag="shifted")
            nc.vector.tensor_add(shifted[:], red_sb[:], nmax.broadcast_to([128, N_PAIRS, 64]))
            expd = sb.tile([128, N_PAIRS, 64], f32, tag="expd")
            nc.scalar.activation(expd[:], shifted[:], mybir.ActivationFunctionType.Exp)
            ssum = sb.tile([128, N_PAIRS, 1], f32, tag="ssum")
            nc.vector.reduce_sum(ssum[:], expd[:], axis=mybir.AxisListType.X)
            rsum = sb.tile([128, N_PAIRS, 1], f32, tag="rsum")
            nc.vector.reciprocal(rsum[:], ssum[:])
            probs = sb.tile([128, N_PAIRS, 64], f32, tag="probs")
            nc.vector.tensor_mul(probs[:], expd[:], rsum.broadcast_to([128, N_PAIRS, 64]))
            probs2 = sb.tile([64, NUM_HEADS, 64], f32, tag="probs2")
            nc.vector.tensor_copy(probs2[:, 0::2, :], probs[0:64, :, :])
            nc.vector.tensor_copy(probs2[:, 1::2, :], probs[64:128, :, :])
            zro = sb.tile([128, N_PAIRS, 128], f32, tag="zro")
            nc.vector.memset(zro[:], 0.0)
            atnT = persist.tile([128, N_PAIRS, 128], f32r)
            nc.vector.tensor_copy(atnT[:], zro[:])

            # ------------- pass B: qT (overlaps the exchange) --------------
            for c in range(N_CHUNKS):
                xt = sb.tile([128, N_CI, CHUNK], f32r, tag="xtb")
                nc.sync.dma_start(xt[:], xT_v[:, :, c * CHUNK:(c + 1) * CHUNK])
                for p in range(N_PAIRS):
                    q_ps = ps_q.tile([128, CHUNK], f32, tag="q")
                    for k in range(N_CI):
                        nc.tensor.matmul(
                            q_ps[:],
                            wq_sb[:, k, p * 128:(p + 1) * 128],
                            xt[:, k, :],
                            start=(k == 0), stop=(k == N_CI - 1),
                        )
                    nc.scalar.copy(qT_all[:, p, c, :], q_ps[:])

        with (
            tc.tile_pool(name="ps_tr", bufs=1, space="PSUM") as ps_tr,
            tc.tile_pool(name="ps_keep", bufs=1, space="PSUM") as ps_keep,
            tc.tile_pool(name="ps_o", bufs=3, space="PSUM") as ps_o,
            tc.tile_pool(name="ps_y", bufs=3, space="PSUM") as ps_y,
        ):
            # HAM keepers in case the exchange outlasts pass B
            keep_ps = ps_keep.tile([128, C], f32)
            for i in range(N_KEEP_MID):
                nc.tensor.matmul(
                    keep_ps[:], ones_sb[:], zrow_sb[:],
                    start=(i == 0), stop=False, skip_group_check=True,
                )

            # ---- transpose probs -> block-diag pair lhsT (f32r) ----
            tr_ps = ps_tr.tile([64, NUM_HEADS, 64], f32)
            for h in range(NUM_HEADS):
                nc.tensor.transpose(tr_ps[:, h, :], probs2[:, h, :], ident[0:64, 0:64])
            for h in range(NUM_HEADS):
                p = h // 2
                off = (h % 2) * 64
                nc.vector.tensor_copy(
                    atnT[off:off + 64, p, off:off + 64], tr_ps[:, h, :]
                )

            # ---------------- phase 2: out + proj --------------------------
            def emit_out(c):
                outT_sb = sb.tile([128, N_CI, CHUNK], f32r, tag="outT", bufs=3, name=f"outT_{c}")
                for p in range(N_PAIRS):
                    o_ps = ps_o.tile([128, CHUNK], f32, tag="o", name=f"o_{c}_{p}")
                    nc.tensor.matmul(
                        o_ps[:], atnT[:, p, :], qT_all[:, p, c, :],
                        start=True, stop=True,
                    )
                    nc.scalar.copy(outT_sb[:, p, :], o_ps[:])
                return outT_sb

            outT_tiles = {0: emit_out(0), 1: emit_out(1)}
            for c in range(N_CHUNKS):
                if c + 2 < N_CHUNKS:
                    outT_tiles[c + 2] = emit_out(c + 2)
                outT_sb = outT_tiles.pop(c)
                for s in range(T_PER_CHUNK):
                    y_ps = ps_y.tile([128, C], f32, tag="y")
                    for k in range(N_CI):
                        nc.tensor.matmul(
                            y_ps[:],
                            outT_sb[:, k, s * TT:(s + 1) * TT],
                            wp_sb[:, k, :],
                            start=(k == 0), stop=(k == N_CI - 1),
                        )
                    y_sb = sb.tile([128, C], f32, tag="ysb", bufs=4)
                    nc.vector.tensor_add(y_sb[:], y_ps[:], bp_f32[:])
                    t0 = c * CHUNK + s * TT
                    nc.sync.dma_start(y[t0:t0 + TT, :], y_sb[:])

    nc.compile()
    return nc


def _get_nc():
    global _NC_CACHE
    if _NC_CACHE is None:
        _NC_CACHE = build_nc()
    return _NC_CACHE


def prep_inputs(x, Wqkv, Wproj, bproj):
    x = np.ascontiguousarray(np.asarray(x, dtype=np.float32))
    Wqkv = np.asarray(Wqkv, dtype=np.float32)
    Wproj = np.asarray(Wproj, dtype=np.float32)
    bproj = np.asarray(bproj, dtype=np.float32)

    xf = x.reshape(B, D * H * W, C)
    wq = np.ascontiguousarray(Wqkv[0:C].T)
    wk = Wqkv[C:2 * C] * np.float32(SCALE)
    wv = Wqkv[2 * C:3 * C]
    wkv = np.ascontiguousarray(np.concatenate([wk, wv], axis=0).T)
    wp = np.ascontiguousarray(Wproj.T)
    bp = np.ascontiguousarray(bproj.reshape(1, C))

    in_maps = []
    for i in range(N_CORES):
        b = i // 2
        t0 = (i % 2) * N_LOC
        own = xf[b, t0:t0 + N_LOC, :]
        pair = xf[b, N_LOC - t0:2 * N_LOC - t0, :]
        xTl = np.ascontiguousarray(np.concatenate([own, pair], axis=0).T)
        in_maps.append({"xT": xTl, "wq": wq, "wkv": wkv, "wp": wp, "bp": bp})
    return in_maps


def gather_output(results):
    parts = [np.asarray(results[i]["y"]) for i in range(N_CORES)]
    return np.concatenate(parts, axis=0).reshape(B, D, H, W, C)


def kernel(x, Wqkv, Wproj, bproj, _trace=False, _tmpdir=None):
    nc = _get_nc()
    in_maps = prep_inputs(x, Wqkv, Wproj, bproj)
    res = run_bass_kernel_spmd(
        nc, in_maps, list(range(N_CORES)), trace=_trace, tmpdir=_tmpdir
    )
    out = gather_output(res.results)
    if _trace:
        kernel.last_exec_time_ns = res.exec_time_ns
        kernel.last_results = res
    return out



# revision 13
# speedup vs baseline: 1.8451x; 1.8451x over previous
"""ChannelAttention (B,D,H,W,C = 4,8,32,32,512; 8 heads, head_dim 64) on 8
Trainium2 NeuronCores, Bass/Tile SPMD. Fully data-parallel: zero cross-core
communication. Cores (2j, 2j+1) handle the two 4096-token halves of batch j.

Algorithm (restructured vs the straightforward qkv formulation):
  k,v are never materialized. The per-head 64x64 Gram matrix is
      G_h = Wk_h S Wv_h^T,   S = X^T X  (over the batch's 8192 tokens),
  and the post-softmax attention is folded into one effective matrix
      Weff^T = sum_h Wq_h^T attn_h^T Wp_h   (built on-chip, tiny),
  so the output pass is a single GEMM  y = X_own @ Weff^T + b.
  This cuts PE work from ~475K matmul rows to ~190K per core.

Schedule per core:
  pass S  : stream x token-major (bf16, 512-token chunks), accumulate the
            upper-triangle blocks of S = X^T X into 4 PSUM banks
            (1280 rows/tile instead of 2048); HAM warm-up keepers run
            during the initial DMA fill. Lower blocks restored by 6 PE
            transposes (S is symmetric).
  sandwich: P = S Wv^T, G_h = Wk_h P_h (pair x quad blocking, f32r --
            the softmax logits are sharp, std ~11, so this path needs
            >= f32r precision), rowwise softmax (DVE/ACT) with PE
            keepers to stay warm.
  fold    : A_h = attn_h Wq_h via transposed probs, Weff^T blocks =
            sum_pairs A_p^T Wp_p, cast to bf16.
  pass y  : stream x^T for the core's own 4096 tokens (bf16), y-tile =
            xT_tile^T @ Weff^T (+bias via DVE broadcast add), DMA out.

Numerics: x streams and Weff in bf16 (measured ~4e-3 end-to-end L2 vs
fp32 reference; budget 2e-2), S/sandwich/fold in f32r with fp32 PSUM.
"""

import numpy as np
import ml_dtypes
from contextlib import ExitStack

import concourse.bass as bass
import concourse.mybir as mybir
import concourse.tile as tile
from concourse import bacc
from concourse.bass_utils import run_bass_kernel_spmd
from concourse.masks import make_identity

B, D, H, W, C = 4, 8, 32, 32, 512
NUM_HEADS = 8
HEAD_DIM = C // NUM_HEADS
SCALE = HEAD_DIM ** -0.5
N_TOK = B * D * H * W
N_CORES = 8
N_LOC = N_TOK // N_CORES          # 4096 tokens per core
N_BATCH = 2 * N_LOC               # 8192 tokens per batch
CHUNK = 512
TT = 128
T_PER_CHUNK = CHUNK // TT
N_CI = C // 128                   # 4 column blocks
N_PAIRS = NUM_HEADS // 2

f32 = mybir.dt.float32
f32r = mybir.dt.float32r
bf16 = mybir.dt.bfloat16

N_KEEP_START = 20
N_KEEP_MID = 10

_NC_CACHE = None


def build_nc():
    nc = bacc.Bacc(num_devices=N_CORES)

    # token-major x for the whole batch (S pass), bf16
    xt_p = nc.declare_dram_parameter("xtok", [N_BATCH, C], bf16, isOutput=False)
    # channel-major x^T for the core's own half (y pass), bf16
    xT_p = nc.declare_dram_parameter("xT", [C, N_LOC], bf16, isOutput=False)
    wkv = nc.declare_dram_parameter("wkv", [C, 2 * C], f32r, isOutput=False)
    wqr = nc.declare_dram_parameter("wqr", [C, C], f32r, isOutput=False)
    wp = nc.declare_dram_parameter("wp", [C, C], f32r, isOutput=False)
    bp = nc.declare_dram_parameter("bp", [1, C], f32r, isOutput=False)
    y = nc.declare_dram_parameter("y", [N_LOC, C], bf16, isOutput=True)

    xt_v = xt_p.rearrange("(a p) c -> p a c", p=128)    # [128, 64, 512]
    xT_v = xT_p.rearrange("(a p) n -> p a n", p=128)    # [128, 4, 4096]
    wkv_v = wkv.rearrange("(a p) f -> p a f", p=128)
    # per-head q rows on partitions 0..63: row h*64+e -> [64(e), 8(h), 512]
    wqr_v = wqr.rearrange("(a p) f -> p a f", p=64)
    wp_v = wp.rearrange("(a p) f -> p a f", p=128)

    N_SCHUNK = N_BATCH // CHUNK   # 16 chunks in the S pass
    N_YCHUNK = N_LOC // CHUNK     # 8 chunks in the y pass

    with tile.TileContext(nc) as tc, ExitStack() as ctx:
        const = ctx.enter_context(tc.tile_pool(name="const", bufs=1))
        persist = ctx.enter_context(tc.tile_pool(name="persist", bufs=1))
        sb = ctx.enter_context(tc.tile_pool(name="sb", bufs=2))

        wkv_sb = const.tile([128, N_CI, 2 * C], f32r)
        nc.sync.dma_start(wkv_sb[:], wkv_v[:])

        ones_f32 = const.tile([1, 128], f32)
        nc.vector.memset(ones_f32[:], 1.0)
        ones_bf = const.tile([1, 128], bf16)
        nc.vector.tensor_copy(ones_bf[:], ones_f32[:])
        zrow_f32 = const.tile([1, 512], f32)
        nc.vector.memset(zrow_f32[:], 0.0)
        zrow_bf = const.tile([1, 512], bf16)
        nc.vector.tensor_copy(zrow_bf[:], zrow_f32[:])
        ident = const.tile([128, 128], f32)
        make_identity(nc, ident[:])
        ident_r = const.tile([128, 128], f32r)
        nc.vector.tensor_copy(ident_r[:], ident[:])

        S_sb = persist.tile([128, N_CI, C], f32r)
        P_sb = persist.tile([128, N_CI, C], f32r)
        A_sb = persist.tile([128, N_PAIRS, C], f32r)
        weffT_sb = persist.tile([128, N_CI, C], bf16)
        red_sb = persist.tile([128, N_PAIRS, 64], f32)
        atnT_sb = persist.tile([64, NUM_HEADS, 64], f32r)

        # triangle block layout: row-block i covers columns i*128..512
        tri_lo = [0, 128, 256, 384]

        keep_pool = ctx.enter_context(tc.tile_pool(name="ps_keep", bufs=1, space="PSUM"))
        keep_ps = keep_pool.tile([128, C], f32)
        keep_i = [0]

        def keepers(n):
            for _ in range(n):
                nc.tensor.matmul(
                    keep_ps[:], ones_bf[:], zrow_bf[:],
                    start=(keep_i[0] == 0), stop=False, skip_group_check=True,
                )
                keep_i[0] += 1

        # ---------------- pass S: S = X^T X (upper triangle) ---------------
        with (
            tc.tile_pool(name="ps_s", bufs=1, space="PSUM") as ps_s,
            tc.tile_pool(name="ps_tr", bufs=2, space="PSUM") as ps_tr,
        ):
            S_ps = ps_s.tile([128, N_CI, C], f32)
            # bank seeds (has_written init) + HAM warm-up during DMA fill
            for i in range(max(N_CI, N_KEEP_START)):
                ci = i % N_CI
                lo = tri_lo[ci]
                nc.tensor.matmul(
                    S_ps[:, ci, lo:C],
                    ones_bf[:], zrow_bf[:, lo:C],
                    start=(i < N_CI), stop=False, skip_group_check=True,
                )

            for c in range(N_SCHUNK):
                xc = sb.tile([128, T_PER_CHUNK, C], bf16, tag="xc", bufs=3)
                nc.sync.dma_start(xc[:], xt_v[:, c * T_PER_CHUNK:(c + 1) * T_PER_CHUNK, :])
                if c == 10:
                    # weights for the later phases; queued behind the x
                    # stream so they don't steal fill bandwidth
                    wqr_sb = const.tile([64, NUM_HEADS, C], f32r)
                    nc.sync.dma_start(wqr_sb[:], wqr_v[:])
                    wp_sb = const.tile([128, N_CI, C], f32r)
                    nc.sync.dma_start(wp_sb[:], wp_v[:])
                    bp_f32 = const.tile([128, C], f32)
                    bp_bcast = bass.AP(
                        tensor=bp[:].bitcast(f32).tensor,
                        offset=0,
                        ap=[[0, 128], [1, C]],
                    )
                    nc.sync.dma_start(bp_f32[:], bp_bcast)
                for t in range(T_PER_CHUNK):
                    xt = xc[:, t, :]
                    last = (c == N_SCHUNK - 1 and t == T_PER_CHUNK - 1)
                    for ci in range(N_CI):
                        lo = tri_lo[ci]
                        nc.tensor.matmul(
                            S_ps[:, ci, lo:C],
                            xt[:, ci * 128:(ci + 1) * 128],
                            xt[:, lo:C],
                            start=False, stop=last, skip_group_check=True,
                        )

            # PSUM -> SBUF (computed triangle); keepers bridge the PE gap
            for ci in range(N_CI):
                lo = tri_lo[ci]
                nc.vector.tensor_copy(S_sb[:, ci, lo:C], S_ps[:, ci, lo:C])
            keepers(4)

            # restore lower blocks by symmetry: S[j,i] = S[i,j]^T
            for i in range(N_CI):
                for j in range(i + 1, N_CI):
                    tr = ps_tr.tile([128, 128], f32r, tag="tr")
                    nc.tensor.transpose(
                        tr[:], S_sb[:, i, j * 128:(j + 1) * 128], ident_r[:]
                    )
                    nc.vector.tensor_copy(S_sb[:, j, i * 128:(i + 1) * 128], tr[:])

        # ---------------- sandwich: P = S Wv^T, G = Wk P ------------------
        with (
            tc.tile_pool(name="ps_p", bufs=2, space="PSUM") as ps_p,
            tc.tile_pool(name="ps_g", bufs=1, space="PSUM") as ps_g,
        ):
            for ablk in range(N_CI):
                p_ps = ps_p.tile([128, C], f32, tag="p")
                for bj in range(N_CI):
                    nc.tensor.matmul(
                        p_ps[:],
                        S_sb[:, bj, ablk * 128:(ablk + 1) * 128],
                        wkv_sb[:, bj, C:2 * C],
                        start=(bj == 0), stop=(bj == N_CI - 1),
                    )
                nc.vector.tensor_copy(P_sb[:, ablk, :], p_ps[:])

            # G pair x quad: out [128 (dk of pair), 256 (de of quad)]
            gq_ps = ps_g.tile([128, N_PAIRS, 256], f32)
            for p in range(N_PAIRS):
                q4 = p // 2
                for aj in range(N_CI):
                    nc.tensor.matmul(
                        gq_ps[:, p, :],
                        wkv_sb[:, aj, p * 128:(p + 1) * 128],
                        P_sb[:, aj, q4 * 256:(q4 + 1) * 256],
                        start=(aj == 0), stop=(aj == N_CI - 1),
                    )
            # extract the 8 useful 64x64 diagonal blocks
            for h in range(NUM_HEADS):
                row0 = (h % 2) * 64
                col0 = (h % 4) * 64
                nc.vector.tensor_copy(
                    red_sb[row0:row0 + 64, h // 2, :],
                    gq_ps[row0:row0 + 64, h // 2, col0:col0 + 64],
                )

        # ---------------- softmax + fold into Weff^T ------------------------
        with (
            tc.tile_pool(name="ps_pt", bufs=1, space="PSUM") as ps_pt,
            tc.tile_pool(name="ps_a", bufs=2, space="PSUM") as ps_a,
            tc.tile_pool(name="ps_w", bufs=2, space="PSUM") as ps_w,
        ):
            # PE keepers so the Tensor engine stays warm through softmax
            keepers(N_KEEP_MID)

            # ---- softmax over e on [128, pair, 64] ----
            nmax = sb.tile([128, N_PAIRS, 1], f32, tag="nmax")
            nc.vector.reduce_max(nmax[:], red_sb[:], axis=mybir.AxisListType.X, negate=True)
            shifted = sb.tile([128, N_PAIRS, 64], f32, tag="shifted")
            nc.vector.tensor_add(shifted[:], red_sb[:], nmax.broadcast_to([128, N_PAIRS, 64]))
            expd = sb.tile([128, N_PAIRS, 64], f32, tag="expd")
            nc.scalar.activation(expd[:], shifted[:], mybir.ActivationFunctionType.Exp)
            ssum = sb.tile([128, N_PAIRS, 1], f32, tag="ssum")
            nc.vector.reduce_sum(ssum[:], expd[:], axis=mybir.AxisListType.X)
            rsum = sb.tile([128, N_PAIRS, 1], f32, tag="rsum")
            nc.vector.reciprocal(rsum[:], ssum[:])
            probs = sb.tile([128, N_PAIRS, 64], f32, tag="probs")
            nc.vector.tensor_mul(probs[:], expd[:], rsum.broadcast_to([128, N_PAIRS, 64]))
            probs2 = sb.tile([64, NUM_HEADS, 64], f32, tag="probs2")
            nc.vector.tensor_copy(probs2[:, 0::2, :], probs[0:64, :, :])
            nc.vector.tensor_copy(probs2[:, 1::2, :], probs[64:128, :, :])

            # transpose probs -> attn^T on partitions 0..63
            tr_ps = ps_pt.tile([64, NUM_HEADS, 64], f32)
            for h in range(NUM_HEADS):
                nc.tensor.transpose(tr_ps[:, h, :], probs2[:, h, :], ident[0:64, 0:64])
            nc.vector.tensor_copy(atnT_sb[:], tr_ps[:])

            # ---- fold: A_h = attn_h Wq_h ; Weff^T = sum_p A_p^T Wp_p ----
            # (matmul dst must start at partition 0: per-head [64, C] tiles,
            # assembled into pair-blocks with partition-shifted DVE copies)
            for h in range(NUM_HEADS):
                a_ps = ps_a.tile([64, C], f32, tag="a")
                nc.tensor.matmul(
                    a_ps[:],
                    atnT_sb[:, h, :],
                    wqr_sb[:, h, :],
                    start=True, stop=True,
                )
                lo = (h % 2) * 64
                nc.vector.tensor_copy(A_sb[lo:lo + 64, h // 2, :], a_ps[:])

            for cblk in range(N_CI):
                w_ps = ps_w.tile([128, C], f32, tag="w")
                for p in range(N_PAIRS):
                    nc.tensor.matmul(
                        w_ps[:],
                        A_sb[:, p, cblk * 128:(cblk + 1) * 128],
                        wp_sb[:, p, :],
                        start=(p == 0), stop=(p == N_PAIRS - 1),
                    )
                nc.vector.tensor_copy(weffT_sb[:, cblk, :], w_ps[:])

        # ---------------- pass y: y = X_own @ Weff^T + b --------------------
        with tc.tile_pool(name="ps_y", bufs=3, space="PSUM") as ps_y:
            for c in range(N_YCHUNK):
                xtc = sb.tile([128, N_CI, CHUNK], bf16, tag="xtc", bufs=2)
                nc.sync.dma_start(xtc[:], xT_v[:, :, c * CHUNK:(c + 1) * CHUNK])
                for t in range(T_PER_CHUNK):
                    y_ps = ps_y.tile([128, C], f32, tag="y")
                    for ci in range(N_CI):
                        nc.tensor.matmul(
                            y_ps[:],
                            xtc[:, ci, t * TT:(t + 1) * TT],
                            weffT_sb[:, ci, :],
                            start=(ci == 0), stop=(ci == N_CI - 1),
                        )
                    y_sb = sb.tile([128, C], bf16, tag="ysb", bufs=4)
                    nc.vector.tensor_add(y_sb[:], y_ps[:], bp_f32[:])
                    t0 = c * CHUNK + t * TT
                    nc.sync.dma_start(y[t0:t0 + TT, :], y_sb[:])

    nc.compile()
    return nc


def _get_nc():
    global _NC_CACHE
    if _NC_CACHE is None:
        _NC_CACHE = build_nc()
    return _NC_CACHE


def prep_inputs(x, Wqkv, Wproj, bproj):
    x = np.ascontiguousarray(np.asarray(x, dtype=np.float32))
    Wqkv = np.asarray(Wqkv, dtype=np.float32)
    Wproj = np.asarray(Wproj, dtype=np.float32)
    bproj = np.asarray(bproj, dtype=np.float32)

    xf = x.reshape(B, N_BATCH, C)
    xf_bf = xf.astype(ml_dtypes.bfloat16)
    wk = Wqkv[C:2 * C] * np.float32(SCALE)
    wv = Wqkv[2 * C:3 * C]
    wkv = np.ascontiguousarray(np.concatenate([wk, wv], axis=0).T)
    wqr = np.ascontiguousarray(Wqkv[0:C])           # q rows x c-in
    wp = np.ascontiguousarray(Wproj.T)
    bpr = np.ascontiguousarray(bproj.reshape(1, C))

    in_maps = []
    for i in range(N_CORES):
        b = i // 2
        t0 = (i % 2) * N_LOC
        xT = np.ascontiguousarray(xf_bf[b, t0:t0 + N_LOC, :].T)
        in_maps.append({
            "xtok": xf_bf[b],
            "xT": xT,
            "wkv": wkv,
            "wqr": wqr,
            "wp": wp,
            "bp": bpr,
        })
    return in_maps


def gather_output(results):
    parts = [np.asarray(results[i]["y"]).astype(np.float32) for i in range(N_CORES)]
    return np.concatenate(parts, axis=0).reshape(B, D, H, W, C)


def kernel(x, Wqkv, Wproj, bproj, _trace=False, _tmpdir=None):
    nc = _get_nc()
    in_maps = prep_inputs(x, Wqkv, Wproj, bproj)
    res = run_bass_kernel_spmd(
        nc, in_maps, list(range(N_CORES)), trace=_trace, tmpdir=_tmpdir
    )
    out = gather_output(res.results)
    if _trace:
        kernel.last_exec_time_ns = res.exec_time_ns
        kernel.last_results = res
    return out
